# revision 5
# baseline (speedup 1.0000x reference)
"""Contrast-depth MSE loss on 8 Trainium2 NeuronCores.

Math: with d = out - label (per image, 32x32 grid flattened to p in [0,1024)),
the loss is an exact quadratic form

    loss = sum_{p,q} C[p,q] * G[p,q] / (B*8*30*30),
    G[p,q] = sum_img d[img,p] * d[img,q]

where C (the contrast-depth-conv quadratic form) is supported on the
diagonals q-p in {0, +-1, +-31, +-32, +-33}.  Each core computes banded
Gram blocks G[128k+r, 128k+c] (c in [0,161)) on the TensorEngine with
PSUM accumulation over its 2048-image shard; the host applies the C
weights to the diagonals and reduces across cores.

Scheduling: the 2048 images are laid on SBUF partitions unevenly to
balance the 16 SDMA engines (engine 15, serving partitions 92-95 and
124-127, is ~18% slower than the rest, so those partitions carry 12
images instead of 16; partitions 0-31 carry 17).  Every image tile is a
contiguous run of input rows, DMA'd per-tile so the vector/tensor
pipeline runs right behind the DMA stream.  The final tile is a
full-width one whose subtract is split at the gram-block boundary so
matmuls and PSUM->SBUF copies (split across the DVE and ACT engines)
start before the subtract finishes.
"""

import numpy as np

_B = 16384
_H = 32
_W = 32
_P = _H * _W  # 1024 pixels
_NCORES = 8
_BSH = _B // _NCORES  # 2048 images per core
_TILE = 128
_BAND = 161  # 128 + max diagonal offset (33)
_NSLOT = 17  # max image-slots per partition


def _block_ncols(k: int) -> int:
    return min(_BAND, _P - 128 * k)


_GRAM_COLS = sum(_block_ncols(k) for k in range(8))  # 7*161 + 128 = 1255


def _build_weights() -> np.ndarray:
    """[128, _GRAM_COLS] weights s.t. loss_sum = sum(W * gram_blocks)."""
    C = np.zeros((_P, _P), dtype=np.float64)
    offs = [(a, b) for a in range(3) for b in range(3) if (a, b) != (1, 1)]
    for a, b in offs:
        for i in range(_H - 2):
            for j in range(_W - 2):
                p = (i + a) * _W + (j + b)  # neighbor pixel
                q = (i + 1) * _W + (j + 1)  # center pixel
                C[p, p] += 1.0
                C[q, q] += 1.0
                C[p, q] -= 1.0
                C[q, p] -= 1.0
    W = np.zeros((_TILE, _GRAM_COLS), dtype=np.float64)
    off = 0
    for k in range(8):
        ncols = _block_ncols(k)
        for delta in (0, 1, 31, 32, 33):
            for r in range(_TILE):
                p = 128 * k + r
                q = p + delta
                c = r + delta
                if q >= _P or c >= ncols:
                    continue
                W[r, off + c] = C[p, q] * (1.0 if delta == 0 else 2.0)
        off += ncols
    return W


_WFULL = _build_weights()


def _tile_table():
    """Program-ordered tiles: (slot, [(p0, p1, row0), ...]).

    Slot counts per partition: p in [0,32): 17, [32,92) and [96,124): 16,
    [92,96) and [124,128): 12.  Each tile's partition runs are contiguous
    input rows, so no host-side rearrangement is needed.
    """
    tiles = []
    # slot 16: the extra image on partitions 0..31 (rows 2016..2048)
    tiles.append((16, [(0, 32, 2016)]))
    # slots 12..15: partitions 0..91 and 96..123 (rows 1536..2016)
    base = 1536
    for s in range(12, 16):
        tiles.append((s, [(0, 92, base), (96, 124, base + 92)]))
        base += 120
    # slots 0..11: all 128 partitions (rows 0..1536); slot 11 is LAST in
    # program order so the tail tile is full-width (single cheap subtract)
    for s in range(12):
        tiles.append((s, [(0, 128, 128 * s)]))
    return tiles


_TILES = _tile_table()
_LAST_SPLIT = 673  # col split of the last tile's subtract: blocks 0-4 | 5-7

_NC_CACHE = None


def _build_nc():
    import concourse.bacc as bacc
    import concourse.mybir as mybir
    import concourse.tile as tile

    nc = bacc.Bacc()
    _FREE = _NSLOT * _P  # 17408 f32 per partition
    out_d = nc.dram_tensor("out", [_BSH, _P], mybir.dt.float32, kind="ExternalInput")
    lab_d = nc.dram_tensor("label", [_BSH, _P], mybir.dt.float32, kind="ExternalInput")
    gram_d = nc.dram_tensor(
        "gram", [_TILE, _GRAM_COLS], mybir.dt.float32, kind="ExternalOutput"
    )

    with tile.TileContext(nc) as tc:
        with (
            tc.tile_pool(name="buf", bufs=1) as buf_pool,
            tc.tile_pool(name="ps", bufs=1, space="PSUM") as psum_pool,
        ):
            grams = []
            offs = []
            off = 0
            for k in range(8):
                ncols = _block_ncols(k)
                grams.append(
                    psum_pool.tile(
                        [_TILE, ncols], mybir.dt.float32, tag=f"g{k}", name=f"g{k}"
                    )
                )
                offs.append(off)
                off += ncols

            # persistent SBUF buffers: every tile DMA can enqueue
            # immediately; no pool-slot rotation ever blocks the DMA stream.
            o = buf_pool.tile([_TILE, _FREE], mybir.dt.float32, tag="o", name="o")
            lb = buf_pool.tile([_TILE, _FREE], mybir.dt.float32, tag="l", name="l")
            d = buf_pool.tile([_TILE, _FREE], mybir.dt.bfloat16, tag="d", name="d")
            result = buf_pool.tile(
                [_TILE, _GRAM_COLS], mybir.dt.float32, tag="r", name="r"
            )

            # the matmul requires base partition in {0,32,64}, so tiles
            # 12-15 contract over all 128 partitions; zero the unloaded
            # "hole" partitions (92-95, 124-127) once so they contribute
            # nothing.  Runs during the DMA ramp, off the critical path.
            hole0, hole1 = 12 * _P, 16 * _P
            nc.vector.memset(d[:, hole0:hole1], 0.0)

            ntiles = len(_TILES)
            for ti, (slot, ranges) in enumerate(_TILES):
                first = ti == 0
                last = ti == ntiles - 1
                c0 = slot * _P
                for p0, p1, r0 in ranges:
                    nrow = p1 - p0
                    nc.sync.dma_start(
                        out=o[p0:p1, c0 : c0 + _P], in_=out_d[r0 : r0 + nrow, :]
                    )
                    nc.scalar.dma_start(
                        out=lb[p0:p1, c0 : c0 + _P], in_=lab_d[r0 : r0 + nrow, :]
                    )
                if not last:
                    for p0, p1, r0 in ranges:
                        nc.vector.tensor_sub(
                            out=d[p0:p1, c0 : c0 + _P],
                            in0=o[p0:p1, c0 : c0 + _P],
                            in1=lb[p0:p1, c0 : c0 + _P],
                        )
                    # single matmul range per tile: full 128 partitions for
                    # the holey tiles (holes are zeroed), [0:32] for slot 16
                    mp1 = 32 if len(ranges) == 1 and ranges[0][1] == 32 else 128
                    for k in range(8):
                        ncols = _block_ncols(k)
                        nc.tensor.matmul(
                            grams[k][:, :ncols],
                            lhsT=d[0:mp1, c0 + 128 * k : c0 + 128 * k + 128],
                            rhs=d[0:mp1, c0 + 128 * k : c0 + 128 * k + ncols],
                            start=first,
                            stop=False,
                        )
                else:
                    # tail tile (full-width): split the subtract at the
                    # block 4/5 boundary so matmuls+copies start early.
                    sp = _LAST_SPLIT
                    nc.vector.tensor_sub(
                        out=d[:, c0 : c0 + sp],
                        in0=o[:, c0 : c0 + sp],
                        in1=lb[:, c0 : c0 + sp],
                    )
                    nc.vector.tensor_sub(
                        out=d[:, c0 + sp : c0 + _P],
                        in0=o[:, c0 + sp : c0 + _P],
                        in1=lb[:, c0 + sp : c0 + _P],
                    )
                    for k in range(8):
                        ncols = _block_ncols(k)
                        nc.tensor.matmul(
                            grams[k][:, :ncols],
                            lhsT=d[:, c0 + 128 * k : c0 + 128 * k + 128],
                            rhs=d[:, c0 + 128 * k : c0 + 128 * k + ncols],
                            start=False,
                            stop=True,
                        )

            # PSUM -> SBUF: blocks 0-4 on the ACT engine, 5-7 on DVE (which
            # is still finishing the tail subtract), then 2 output DMAs.
            for k in range(8):
                ncols = _block_ncols(k)
                dst = result[:, offs[k] : offs[k] + ncols]
                if k < 5:
                    nc.scalar.copy(out=dst, in_=grams[k][:])
                else:
                    nc.vector.tensor_copy(out=dst, in_=grams[k][:])
            split = offs[5]
            nc.sync.dma_start(out=gram_d[:, :split], in_=result[:, :split])
            nc.sync.dma_start(out=gram_d[:, split:], in_=result[:, split:])
    nc.finalize()
    return nc


def _run(out, label, trace=False):
    from concourse.bass_utils import run_bass_kernel_spmd

    global _NC_CACHE
    out = np.ascontiguousarray(np.asarray(out), dtype=np.float32).reshape(_B, _P)
    label = np.ascontiguousarray(np.asarray(label), dtype=np.float32).reshape(_B, _P)
    if _NC_CACHE is None:
        _NC_CACHE = _build_nc()
    in_maps = [
        {
            "out": out[i * _BSH : (i + 1) * _BSH],
            "label": label[i * _BSH : (i + 1) * _BSH],
        }
        for i in range(_NCORES)
    ]
    res = run_bass_kernel_spmd(
        _NC_CACHE, in_maps, core_ids=list(range(_NCORES)), trace=trace
    )
    total = 0.0
    for r in res.results:
        total += float((_WFULL * r["gram"].astype(np.float64)).sum())
    loss = total / (_B * 8 * (_H - 2) * (_W - 2))
    return np.asarray(np.float32(loss)), res


def kernel(out, label):
    loss, _ = _run(out, label, trace=False)
    return loss
